# revision 27
# baseline (speedup 1.0000x reference)
"""Trainium2 Bass kernel for nn_DPR (dense_transformer), v2.

Distribution: batch-parallel over 8 NeuronCores (same sharding as v1).
Math identical to v1 (see reference): q/k projections, view-scrambled
attention, per-192-segment softmax, PV, fc + residual + layernorm.

v2 changes vs the v1 baseline (305us/iter on HW):
- All staged/derived matmul data is bf16: halves collective bytes, DMA
  traffic, SBUF footprint, and PE transpose cost (1.0 vs 2.0 cyc/row).
  fp32 inputs are converted once on DVE/Pool right after load.
- The k AllGather stays separate and hides under the q phase.
- a2a chunk layout [bl, b, j, d]: the (dest-core, dest-row-block)
  partition structure merges to one stride-12288 dim on both sides, so
  staging is 16 DMAs of 48KB (two 4-batch waves) and the attention lhsT
  loads are 8 DMAs, all within the 3-dim DMA AP limit.
- DMA triggers only on SP/ACT HWDGE queues (plus a few SWDGE on Pool for
  off-critical loads); ACT's queue is kept clear during softmax.
- k projection matmul split 288/288 so both halves hit the >=256-row
  fp32r/bf16 fast path.
- DRAM temps are double-buffered by repeat parity so back-to-back
  iterations in one NEFF can pipeline.
"""
import os
import numpy as np
from contextlib import ExitStack

import concourse.bass as bass
import concourse.tile as tile
from concourse import mybir, bacc
from concourse.bass_utils import run_bass_kernel_spmd
from concourse.masks import make_identity

F32 = mybir.dt.float32
F32R = mybir.dt.float32r
BF16 = mybir.dt.bfloat16
ExpF = mybir.ActivationFunctionType.Exp
SqrtF = mybir.ActivationFunctionType.Sqrt
MULT = mybir.AluOpType.mult
ADD = mybir.AluOpType.add

NCORES = 8
FEAT = 192          # d
PTS = 2048          # n
PROJ = 512          # p
BATCH = 64
NSUP = 5
WAY = 6
BL = BATCH // NCORES            # 8 local batches
GL = BL * WAY                   # 48 local (b, w) rows
TEMP = float(FEAT) ** 0.5
LN_EPS = 1e-5
NKT = PTS // 128                # 16 contraction tiles for projections
MT = BL * FEAT // 128           # 12 attention row tiles (1536 rows)
NCOL = WAY * FEAT               # 1152 attention cols
NS = NSUP * FEAT
KPART = 128 * 576               # k AllGather piece (elems)

# a2a chunk layout per (src->dest): [bl 8, b 8, j 8, d 192]
# strides within a chunk: bl -> 12288, b -> 1536, j -> 192, d -> 1
QCH = 8 * 8 * 8 * FEAT          # 98304 elems per chunk


def _ap(t, offset, dims):
    return bass.AP(tensor=t.tensor, offset=t.offset + offset, ap=list(dims))


def build(repeat=None):
    if repeat is None:
        repeat = int(os.environ.get("KERNEL_REPEAT", "1"))
    skip_cc = bool(os.environ.get("KERNEL_SKIP_CC"))
    nc = bacc.Bacc("TRN2", target_bir_lowering=False, debug=False,
                   num_devices=NCORES)

    query = nc.dram_tensor("query", [BL, FEAT, PTS], F32, kind="ExternalInput").ap()
    sup_sh = nc.dram_tensor("sup_sh", [3, FEAT, PTS], F32, kind="ExternalInput").ap()
    proto = nc.dram_tensor("proto", [GL, FEAT], F32, kind="ExternalInput").ap()
    Wq = nc.dram_tensor("Wq", [PROJ, PTS], F32, kind="ExternalInput").ap()
    Wk_sh = nc.dram_tensor("Wk_sh", [128, PTS], F32, kind="ExternalInput").ap()
    Wv = nc.dram_tensor("Wv", [FEAT, FEAT], F32, kind="ExternalInput").ap()
    bv = nc.dram_tensor("bv", [FEAT], F32, kind="ExternalInput").ap()
    Wf = nc.dram_tensor("Wf", [FEAT, FEAT], F32, kind="ExternalInput").ap()
    ln_g = nc.dram_tensor("ln_g", [FEAT], F32, kind="ExternalInput").ap()
    ln_b = nc.dram_tensor("ln_b", [FEAT], F32, kind="ExternalInput").ap()
    out_l = nc.dram_tensor("out_l", [GL, FEAT], F32, kind="ExternalOutput").ap()

    # per-parity DRAM temps
    temps = []
    for par in range(2 if repeat > 1 else 1):
        P = f"_p{par}"
        temps.append(dict(
            a2a_in=nc.dram_tensor(f"a2a_in{P}", [NCORES, QCH], BF16),
            a2a_out=nc.dram_tensor(f"a2a_out{P}", [NCORES, QCH], BF16),
            kag_in=nc.dram_tensor(f"kag_in{P}", [KPART], BF16),
            kag_out=nc.dram_tensor(f"kag_out{P}", [NCORES, KPART], BF16,
                                   addr_space="Shared"),
            kflat=nc.dram_tensor(f"kflat{P}", [WAY * PROJ, FEAT], BF16),
            vflat=nc.dram_tensor(f"vflat{P}", [GL, FEAT], BF16),
            opre_d=nc.dram_tensor(f"opre_d{P}", [MT * 128, WAY], F32),
        ))

    with tile.TileContext(nc) as tc:
        for rep in range(repeat):
            _emit(nc, tc, rep, skip_cc, temps[rep % len(temps)],
                  query, sup_sh, proto, Wq, Wk_sh, Wv, bv, Wf, ln_g, ln_b,
                  out_l)

    nc.compile()
    return nc


def _emit(nc, tc, rep, skip_cc, T,
          query, sup_sh, proto, Wq, Wk_sh, Wv, bv, Wf, ln_g, ln_b, out_l):
    R = f"r{rep}"
    a2a_in, a2a_out = T["a2a_in"], T["a2a_out"]
    kag_in, kag_out = T["kag_in"], T["kag_out"]
    kflat, vflat, opre_d = T["kflat"], T["vflat"], T["opre_d"]

    with ExitStack() as top:
        const = top.enter_context(tc.tile_pool(name=f"const{R}", bufs=1))
        ident = const.tile([128, 128], F32, name=f"ident{R}")
        make_identity(nc, ident)
        identb = const.tile([128, 128], BF16, name=f"identb{R}")
        make_identity(nc, identb)
        proto_sb = const.tile([GL, FEAT], F32, name=f"proto_sb{R}")
        nc.scalar.dma_start(proto_sb, proto)

        # ============ v = proto @ Wv.T + bv (tiny, local) ============
        with ExitStack() as ctx:
            vp = ctx.enter_context(tc.tile_pool(name=f"vp{R}", bufs=1))
            vps = ctx.enter_context(tc.tile_pool(name=f"vps{R}", bufs=2, space="PSUM"))

            wv_hi = vp.tile([128, FEAT], F32, tag="wn")
            wv_lo = vp.tile([64, FEAT], F32, tag="wn2")
            nc.scalar.dma_start(wv_hi, Wv[0:128, :])
            nc.scalar.dma_start(wv_lo, Wv[128:192, :])

            wvT_hi = vp.tile([128, FEAT], F32R, tag="wt")
            wvT_lo = vp.tile([64, FEAT], F32R, tag="wt2")
            ptT_hi = vp.tile([128, GL], F32R, tag="pt")
            ptT_lo = vp.tile([64, GL], F32R, tag="pt2")
            for (dst, dcol, src) in (
                (wvT_hi, slice(0, 128), wv_hi[:, 0:128]),
                (wvT_lo, slice(0, 128), wv_hi[:, 128:192]),
                (wvT_hi, slice(128, 192), wv_lo[:, 0:128]),
                (wvT_lo, slice(128, 192), wv_lo[:, 128:192]),
                (ptT_hi, slice(0, GL), proto_sb[:, 0:128]),
                (ptT_lo, slice(0, GL), proto_sb[:, 128:192]),
            ):
                p_in, f_in = src.shape
                ps_t = vps.tile([128, 128], F32, tag="tp")
                nc.tensor.transpose(ps_t[:f_in, :p_in], src, ident[:p_in, :p_in])
                nc.vector.tensor_copy(dst[:f_in, dcol], ps_t[:f_in, :p_in])

            ps_v = vps.tile([GL, FEAT], F32, tag="v")
            nc.tensor.matmul(ps_v, ptT_hi, wvT_hi, start=True, stop=False)
            nc.tensor.matmul(ps_v, ptT_lo, wvT_lo, start=False, stop=True)
            bv_bc = vp.tile([GL, FEAT], F32, tag="bv")
            nc.scalar.dma_start(bv_bc, _ap(bv, 0, [[0, GL], [1, FEAT]]))
            v_sb = vp.tile([GL, FEAT], BF16, tag="vs")
            nc.vector.tensor_add(v_sb, ps_v, bv_bc)
            nc.scalar.dma_start(vflat.ap(), v_sb)

        # ====== k projection: 8-way sharded (pt = core%4, ways = core//4) ======
        # bf16: convert sup/wk after load, transpose at 1 cyc/row, AG bf16.
        with ExitStack() as ctxk:
            kn = ctxk.enter_context(tc.tile_pool(name=f"kn{R}", bufs=2))
            kb = ctxk.enter_context(tc.tile_pool(name=f"kb{R}", bufs=1))
            ktp = ctxk.enter_context(tc.tile_pool(name=f"ktp{R}", bufs=1))
            kps = ctxk.enter_context(tc.tile_pool(name=f"kps{R}", bufs=1, space="PSUM"))
            tps2 = ctxk.enter_context(tc.tile_pool(name=f"tps2{R}", bufs=2, space="PSUM"))
            kev = ctxk.enter_context(tc.tile_pool(name=f"kev{R}", bufs=1))

            wk_nat = kn.tile([128, PTS], F32, tag="wkn", name=f"wkn{R}")
            nc.sync.dma_start(wk_nat, Wk_sh)
            wk_b = kb.tile([128, PTS], BF16, tag="wkb", name=f"wkb{R}")
            nc.gpsimd.tensor_copy(wk_b, wk_nat)
            sup_b = []
            for wi in range(3):
                hi = kn.tile([128, PTS], F32, tag="shi", name=f"shi{R}{wi}")
                lo = kn.tile([64, PTS], F32, tag="slo", name=f"slo{R}{wi}")
                nc.sync.dma_start(hi, sup_sh[wi, 0:128, :])
                nc.sync.dma_start(lo, sup_sh[wi, 128:192, :])
                bhi = kb.tile([128, PTS], BF16, tag=f"sbh{wi}", name=f"sbh{R}{wi}")
                blo = kb.tile([64, PTS], BF16, tag=f"sbl{wi}", name=f"sbl{R}{wi}")
                nc.gpsimd.tensor_copy(bhi, hi)
                nc.gpsimd.tensor_copy(blo, lo)
                sup_b.append((bhi, blo))

            wkT = [ktp.tile([128, 128], BF16, tag=f"wkt{kt}", name=f"wkT{R}{kt}")
                   for kt in range(NKT)]
            supT = [ktp.tile([128, 3 * FEAT], BF16, tag=f"st{kt}", name=f"supT{R}{kt}")
                    for kt in range(NKT)]
            for kt in range(NKT):
                ksl = slice(128 * kt, 128 * (kt + 1))
                # pack wk + 3x(hi/lo) transposes into one [128, 704] psum
                ps_t = tps2.tile([128, 704], BF16, tag="tp")
                nc.tensor.transpose(ps_t[:, 0:128], wk_b[:, ksl], identb)
                for wi in range(3):
                    bhi, blo = sup_b[wi]
                    c0 = 128 + wi * FEAT
                    nc.tensor.transpose(ps_t[:, c0:c0 + 128], bhi[:, ksl], identb)
                    nc.tensor.transpose(ps_t[:, c0 + 128:c0 + 192], blo[:, ksl],
                                        identb[0:64, 0:64])
                nc.vector.tensor_copy(wkT[kt], ps_t[:, 0:128])
                nc.vector.tensor_copy(supT[kt], ps_t[:, 128:704])

            ps_ka = kps.tile([128, 288], F32, tag="ka")
            ps_kb = kps.tile([128, 288], F32, tag="kb")
            for kt in range(NKT):
                nc.tensor.matmul(ps_ka, wkT[kt], supT[kt][:, 0:288],
                                 start=(kt == 0), stop=(kt == NKT - 1))
                nc.tensor.matmul(ps_kb, wkT[kt], supT[kt][:, 288:576],
                                 start=(kt == 0), stop=(kt == NKT - 1))
            k_part = kev.tile([128, 576], BF16, tag="kpart")
            nc.scalar.copy(k_part[:, 0:288], ps_ka)
            nc.scalar.copy(k_part[:, 288:576], ps_kb)
            nc.sync.dma_start(
                _ap(kag_in.ap(), 0, [[576, 128], [1, 576]]), k_part)

        if not skip_cc:
            nc.gpsimd.collective_compute(
                "AllGather", mybir.AluOpType.bypass,
                replica_groups=[list(range(NCORES))],
                ins=[kag_in.ap()], outs=[kag_out.ap()])

        # ============ WqT (scaled 1/TEMP) + q projection + A2A staging =====
        with ExitStack() as ctxq:
            wqp = ctxq.enter_context(tc.tile_pool(name=f"wqp{R}", bufs=2))
            wqb = ctxq.enter_context(tc.tile_pool(name=f"wqb{R}", bufs=1))
            wqtp = ctxq.enter_context(tc.tile_pool(name=f"wqtp{R}", bufs=1))
            tps = ctxq.enter_context(tc.tile_pool(name=f"tps{R}", bufs=2, space="PSUM"))

            wq_b = []
            for pt in range(4):
                t = wqp.tile([128, PTS], F32, tag=f"wqn{pt % 2}", name=f"wqn{R}{pt}")
                nc.scalar.dma_start(t, Wq[128 * pt:128 * (pt + 1), :])
                tb = wqb.tile([128, PTS], BF16, tag=f"wqb{pt}", name=f"wqb{R}{pt}")
                nc.vector.tensor_copy(tb, t)
                wq_b.append(tb)
            wqT = [wqtp.tile([128, PROJ], BF16, tag=f"wqt{kt}", name=f"wqT{R}{kt}")
                   for kt in range(NKT)]
            for kt in range(NKT):
                ps_t = tps.tile([128, 512], BF16, tag="tp")
                for pt in range(4):
                    nc.tensor.transpose(
                        ps_t[:, 128 * pt:128 * (pt + 1)],
                        wq_b[pt][:, 128 * kt:128 * (kt + 1)], identb)
                nc.scalar.mul(wqT[kt], ps_t, 1.0 / TEMP)

            qn = ctxq.enter_context(tc.tile_pool(name=f"qn{R}", bufs=3))
            qbp = ctxq.enter_context(tc.tile_pool(name=f"qbp{R}", bufs=2))
            qtp = ctxq.enter_context(tc.tile_pool(name=f"qtp{R}", bufs=2))
            qps = ctxq.enter_context(tc.tile_pool(name=f"qps{R}", bufs=2, space="PSUM"))
            qsp = ctxq.enter_context(tc.tile_pool(name=f"qsp{R}", bufs=2))

            # 4-batch waves: transposes per 2-batch pair, one 768-wide
            # bf16 matmul per (kt, pt) per wave (halves matmul+ldweights
            # dispatch), evac once per (pt, wave), stage 8 DMAs per wave.
            for wv in range(2):
                qt4 = [qtp.tile([128, 4 * FEAT], BF16, tag=f"qt{kt}",
                                name=f"qt4{R}{wv}{kt}") for kt in range(NKT)]
                for hp in range(2):
                    b0 = 4 * wv + 2 * hp
                    natb = []
                    for b in (b0, b0 + 1):
                        hi = qn.tile([128, PTS], F32, tag="qhi", name=f"qhi{R}{b}")
                        lo = qn.tile([64, PTS], F32, tag="qlo", name=f"qlo{R}{b}")
                        nc.sync.dma_start(hi, query[b, 0:128, :])
                        nc.sync.dma_start(lo, query[b, 128:192, :])
                        bhi = qbp.tile([128, PTS], BF16, tag="qbh", name=f"qbh{R}{b}")
                        blo = qbp.tile([64, PTS], BF16, tag="qbl", name=f"qbl{R}{b}")
                        nc.gpsimd.tensor_copy(bhi, hi)
                        nc.gpsimd.tensor_copy(blo, lo)
                        natb.append((bhi, blo))
                    for kt in range(NKT):
                        ksl = slice(128 * kt, 128 * (kt + 1))
                        ps_t = tps.tile([128, 2 * FEAT], BF16, tag="tp")
                        for bi in range(2):
                            bhi, blo = natb[bi]
                            nc.tensor.transpose(
                                ps_t[:, bi * FEAT:bi * FEAT + 128],
                                bhi[:, ksl], identb)
                            nc.tensor.transpose(
                                ps_t[:, bi * FEAT + 128:bi * FEAT + 192],
                                blo[:, ksl], identb[0:64, 0:64])
                        nc.vector.tensor_copy(
                            qt4[kt][:, hp * 2 * FEAT:(hp + 1) * 2 * FEAT], ps_t)
                # pt-outer accumulation: one rotating [128, 768] psum.
                # evac (bf16) + stage. psum partition i of ps_q is
                # p = 128pt + 64h + 8ch + bl; free is (b-in-wave, d).
                # chunk layout [bl, b, j, d], j = 2pt+h; partition (ch, bl)
                # maps to dest offset i*12288 (merged dim).
                w0 = 4 * wv
                for pt in range(4):
                    ps_q = qps.tile([128, 4 * FEAT], F32, tag="q",
                                    name=f"psq{R}{wv}{pt}")
                    for kt in range(NKT):
                        # matmul out must stay within one 512-f32 PSUM bank
                        lw = wqT[kt][:, 128 * pt:128 * (pt + 1)]
                        nc.tensor.matmul(
                            ps_q[:, 0:512], lw, qt4[kt][:, 0:512],
                            start=(kt == 0), stop=(kt == NKT - 1))
                        nc.tensor.matmul(
                            ps_q[:, 512:768], lw, qt4[kt][:, 512:768],
                            start=(kt == 0), stop=(kt == NKT - 1))
                    q_sb = qsp.tile([128, 4 * FEAT], BF16, tag=f"qsb{pt}",
                                    name=f"qsb{R}{wv}{pt}")
                    nc.scalar.copy(q_sb, ps_q)
                    for h in range(2):
                        j = 2 * pt + h
                        dst = _ap(a2a_in.ap(),
                                  j * FEAT + w0 * (8 * FEAT),
                                  [[12288, 64], [8 * FEAT, 4], [1, FEAT]])
                        eng = nc.sync if (pt % 2 == 0) else nc.scalar
                        eng.dma_start(dst, q_sb[64 * h:64 * h + 64, :])

        # ====== kflat reassembly from AllGathered pieces (bf16) ======
        with ExitStack() as ctka:
            kap = ctka.enter_context(tc.tile_pool(name=f"kap{R}", bufs=2))
            for pt in range(4):
                a0 = kap.tile([128, 576], BF16, tag="a0")
                a1 = kap.tile([128, 384], BF16, tag="a1")
                nc.sync.dma_start(
                    a0, _ap(kag_out.ap(), pt * KPART, [[576, 128], [1, 576]]))
                nc.sync.dma_start(
                    a1, _ap(kag_out.ap(), (pt + 4) * KPART, [[576, 128], [1, 384]]))
                k_sb = kap.tile([128, NCOL], BF16, tag="ksb")
                nc.vector.tensor_copy(k_sb[:, FEAT:4 * FEAT], a0)
                nc.vector.tensor_copy(k_sb[:, 4 * FEAT:NCOL], a1)
                k0a = kap.tile([128, FEAT], BF16, tag="k0a")
                k0b = kap.tile([128, FEAT], BF16, tag="k0b")
                nc.vector.tensor_add(k0a, k_sb[:, FEAT:2 * FEAT],
                                     k_sb[:, 2 * FEAT:3 * FEAT])
                nc.vector.tensor_add(k0b, k_sb[:, 3 * FEAT:4 * FEAT],
                                     k_sb[:, 4 * FEAT:5 * FEAT])
                nc.vector.tensor_add(k0a, k0a, k0b)
                nc.vector.tensor_add(k0a, k0a, k_sb[:, 5 * FEAT:6 * FEAT])
                nc.vector.tensor_scalar_mul(k_sb[:, 0:FEAT], k0a, 1.0 / NSUP)
                nc.scalar.dma_start(
                    _ap(kflat.ap(), (128 * pt) * FEAT,
                        [[FEAT, 128], [PROJ * FEAT, WAY], [1, FEAT]]),
                    k_sb)

        # ============ AllToAll q exchange ============
        if not skip_cc:
            nc.gpsimd.collective_compute(
                "AllToAll", mybir.AluOpType.bypass,
                replica_groups=[list(range(NCORES))],
                ins=[a2a_in.ap()], outs=[a2a_out.ap()])

        # ============ attention + softmax + PV ============
        with ExitStack() as ctxa:
            kfp = ctxa.enter_context(tc.tile_pool(name=f"kfp{R}", bufs=1))
            lhp = ctxa.enter_context(tc.tile_pool(name=f"lhp{R}", bufs=1))
            aps = ctxa.enter_context(tc.tile_pool(name=f"aps{R}", bufs=2, space="PSUM"))
            ep = ctxa.enter_context(tc.tile_pool(name=f"ep{R}", bufs=4))
            sp = ctxa.enter_context(tc.tile_pool(name=f"sp{R}", bufs=6))
            scp = ctxa.enter_context(tc.tile_pool(name=f"scp{R}", bufs=6))

            kf = []
            for kt4 in range(4):
                t_r = kfp.tile([128, NCOL], BF16, tag=f"kf{kt4}", name=f"kfr{R}{kt4}")
                nc.scalar.dma_start(
                    t_r, _ap(kflat.ap(), kt4 * 128 * NCOL,
                             [[NCOL, 128], [1, NCOL]]))
                kf.append(t_r)
            # preload all replicated v tiles so no DMA sits on ACT/Pool
            # queues during the softmax pipeline
            vbs = []
            for mt in range(MT):
                vb = kfp.tile([128, FEAT], BF16, tag=f"vb{mt}", name=f"vb{R}{mt}")
                eng = nc.scalar if (mt % 2) else nc.sync
                eng.dma_start(
                    vb, _ap(vflat.ap(), mt * 4 * FEAT,
                            [[FEAT, 4], [0, 32], [1, FEAT]]))
                vbs.append(vb)

            # lhsT tiles: [128 r, 1536 m]; tile partitions are (sc, b, j)
            # which merge to a single stride-192 dim in the chunk; free m
            # is (bl -> 12288, d -> 1).
            lhs_r = []
            for kt4 in range(4):
                l_r = lhp.tile([128, BL * FEAT], BF16, tag=f"lh{kt4}",
                               name=f"lh{R}{kt4}")
                for sc in range(2):
                    nc.sync.dma_start(
                        l_r[64 * sc:64 * sc + 64, :],
                        _ap(a2a_out.ap(), (2 * kt4 + sc) * QCH,
                            [[FEAT, 64], [12288, 8], [1, FEAT]]))
                lhs_r.append(l_r)

            for mt in range(MT):
                ps_at = [aps.tile([128, 2 * FEAT], F32, tag=f"at{nch}",
                                  name=f"psat{R}{mt}{nch}") for nch in range(3)]
                msl = slice(128 * mt, 128 * (mt + 1))
                for kt4 in range(4):
                    for nch in range(3):
                        nc.tensor.matmul(
                            ps_at[nch], lhs_r[kt4][:, msl],
                            kf[kt4][:, 384 * nch:384 * (nch + 1)],
                            start=(kt4 == 0), stop=(kt4 == 3))

                # exp without max-subtraction (logits are O(7); safe in f32)
                e6 = ep.tile([128, NCOL], BF16, tag="e6")
                sums = sp.tile([128, WAY], F32, tag="sums")
                for nch in range(3):
                    for s2 in range(2):
                        s = 2 * nch + s2
                        seg = ps_at[nch][:, s2 * FEAT:(s2 + 1) * FEAT]
                        nc.scalar.activation(
                            e6[:, s * FEAT:(s + 1) * FEAT], seg, ExpF,
                            accum_out=sums[:, s:s + 1])

                opre = sp.tile([128, WAY], F32, tag="opre")
                for s in range(WAY):
                    scr = scp.tile([128, FEAT], BF16, tag="scr")
                    nc.vector.scalar_tensor_tensor(
                        out=scr, in0=e6[:, s * FEAT:(s + 1) * FEAT],
                        scalar=1.0, in1=vbs[mt], op0=MULT, op1=MULT,
                        accum_out=opre[:, s:s + 1])
                rec = sp.tile([128, WAY], F32, tag="rec")
                nc.vector.reciprocal(rec, sums)
                nc.vector.tensor_mul(opre, opre, rec)
                nc.sync.dma_start(
                    _ap(opre_d.ap(), mt * 128 * WAY, [[WAY, 128], [1, WAY]]),
                    opre)

        # ============ output head: fc + residual + layernorm ============
        with ExitStack() as ctxo:
            fp = ctxo.enter_context(tc.tile_pool(name=f"fp{R}", bufs=1))
            fps = ctxo.enter_context(tc.tile_pool(name=f"fps{R}", bufs=2, space="PSUM"))

            wf_hi = fp.tile([128, FEAT], F32, tag="wfn")
            wf_lo = fp.tile([64, FEAT], F32, tag="wfn2")
            nc.sync.dma_start(wf_hi, Wf[0:128, :])
            nc.sync.dma_start(wf_lo, Wf[128:192, :])
            wfT_hi = fp.tile([128, FEAT], F32R, tag="wft")
            wfT_lo = fp.tile([64, FEAT], F32R, tag="wft2")
            op_sb = fp.tile([GL, FEAT], F32, tag="opsb")
            nc.sync.dma_start(op_sb, _ap(opre_d.ap(), 0, [[FEAT, GL], [1, FEAT]]))
            opT_hi = fp.tile([128, GL], F32R, tag="opt")
            opT_lo = fp.tile([64, GL], F32R, tag="opt2")
            for (dst, dcol, src) in (
                (wfT_hi, slice(0, 128), wf_hi[:, 0:128]),
                (wfT_lo, slice(0, 128), wf_hi[:, 128:192]),
                (wfT_hi, slice(128, 192), wf_lo[:, 0:128]),
                (wfT_lo, slice(128, 192), wf_lo[:, 128:192]),
                (opT_hi, slice(0, GL), op_sb[:, 0:128]),
                (opT_lo, slice(0, GL), op_sb[:, 128:192]),
            ):
                p_in, f_in = src.shape
                ps_t = fps.tile([128, 128], F32, tag="tp")
                nc.tensor.transpose(ps_t[:f_in, :p_in], src, ident[:p_in, :p_in])
                nc.vector.tensor_copy(dst[:f_in, dcol], ps_t[:f_in, :p_in])

            ps_o = fps.tile([GL, FEAT], F32, tag="o2")
            nc.tensor.matmul(ps_o, opT_hi, wfT_hi, start=True, stop=False)
            nc.tensor.matmul(ps_o, opT_lo, wfT_lo, start=False, stop=True)

            x_sb = fp.tile([GL, FEAT], F32, tag="x")
            nc.vector.tensor_add(x_sb, ps_o, proto_sb)
            st = fp.tile([GL, 6], F32, tag="st")
            nc.vector.bn_stats(st, x_sb)
            mv = fp.tile([GL, 2], F32, tag="mv")
            nc.vector.bn_aggr(mv, st)
            eps_t = fp.tile([GL, 1], F32, tag="eps")
            nc.vector.memset(eps_t, LN_EPS)
            std = fp.tile([GL, 1], F32, tag="std")
            nc.scalar.activation(std, mv[:, 1:2], SqrtF, bias=eps_t, scale=1.0)
            rstd = fp.tile([GL, 1], F32, tag="rstd")
            nc.vector.reciprocal(rstd, std)
            negmean = fp.tile([GL, 1], F32, tag="nm")
            nc.scalar.mul(negmean, mv[:, 0:1], -1.0)
            y = fp.tile([GL, FEAT], F32, tag="y")
            nc.vector.tensor_scalar(
                out=y, in0=x_sb, scalar1=negmean, scalar2=rstd,
                op0=ADD, op1=MULT)
            lng_bc = fp.tile([GL, FEAT], F32, tag="lg")
            lnb_bc = fp.tile([GL, FEAT], F32, tag="lb")
            nc.scalar.dma_start(lng_bc, _ap(ln_g, 0, [[0, GL], [1, FEAT]]))
            nc.scalar.dma_start(lnb_bc, _ap(ln_b, 0, [[0, GL], [1, FEAT]]))
            nc.vector.tensor_mul(y, y, lng_bc)
            nc.vector.tensor_add(y, y, lnb_bc)
            nc.sync.dma_start(out_l, y)


_NC = None


def kernel(query, support, prototype, Wq, Wk, Wv, bv, Wf, ln_g, ln_b,
           _trace=False):
    global _NC
    if _NC is None:
        _NC = build()
    query = np.ascontiguousarray(np.asarray(query, np.float32))
    prototype = np.ascontiguousarray(np.asarray(prototype, np.float32))
    support = np.asarray(support, np.float32)
    Wk = np.asarray(Wk, np.float32)
    shared = {
        "Wq": np.ascontiguousarray(np.asarray(Wq, np.float32)),
        "Wv": np.ascontiguousarray(np.asarray(Wv, np.float32)),
        "bv": np.ascontiguousarray(np.asarray(bv, np.float32)),
        "Wf": np.ascontiguousarray(np.asarray(Wf, np.float32)),
        "ln_g": np.ascontiguousarray(np.asarray(ln_g, np.float32)),
        "ln_b": np.ascontiguousarray(np.asarray(ln_b, np.float32)),
    }
    zway = np.zeros((1, FEAT, PTS), np.float32)
    sup_h0 = np.ascontiguousarray(support[0:3])
    sup_h1 = np.ascontiguousarray(np.concatenate([support[3:5], zway], 0))
    pf = prototype.reshape(BATCH * WAY, FEAT)
    in_maps = []
    for c in range(NCORES):
        pt, half = c % 4, c // 4
        in_maps.append({
            "query": np.ascontiguousarray(query[c * BL:(c + 1) * BL]),
            "proto": np.ascontiguousarray(pf[c * GL:(c + 1) * GL]),
            "sup_sh": sup_h0 if half == 0 else sup_h1,
            "Wk_sh": np.ascontiguousarray(Wk[128 * pt:128 * (pt + 1)]),
            **shared,
        })
    res = run_bass_kernel_spmd(_NC, in_maps, list(range(NCORES)),
                               trace=_trace)
    out = np.concatenate([res.results[c]["out_l"] for c in range(NCORES)], 0)
    out = out.reshape(BATCH, WAY, FEAT)
    if _trace:
        return out, res
    return out
